# revision 4
# baseline (speedup 1.0000x reference)
"""Trainium2 Bass kernel for nn_MultiHeadedAttention (B=4, S=1024, D=1024, H=16).

Sharding: 8 cores = 4 batches x 2 head-halves (8 heads each). The reference's
row-major reshape after [B,H,S,d] means output row r = h*64 + s//16 depends
only on head h, so head sharding needs no collective: each core computes a
[512, 1024] row-block of its batch's output.

Per-core pipeline (all matmuls contract on the partition dim):
  QT/KT = WxT.T @ XxT          -> [j, s] layout (head dims on partitions)
  V     = XvT.T @ WvT          -> [s, j] natural layout, augmented with a
                                  ones column per head (row 64 of PV psum
                                  then accumulates the softmax denominator)
  scoresT[k, q] = KT_h.T @ QT_h  (q in s16-major order so PV output lands in
                                  the layout the final reshape needs)
  wT = exp(0.125 * scoresT)      (mask is a no-op unless mask@mask.T has
                                  zeros; host checks and enables a penalty-add
                                  fallback path in that case)
  xT'[dd|sum, q] = V_aug.T @ wT  (accumulated over k tiles)
  lhsT = xT'[0:64] * (1/sum)     (DVE copy into x_block.T layout, 2 heads
                                  side by side)
  out  = lhsT.T @ WoT            -> [128 rows, 1024] per head pair, DMA'd out.
"""

import numpy as np

import concourse.bass as bass
import concourse.bacc as bacc
import concourse.tile as tile
from concourse import mybir
from concourse.bass_utils import run_bass_kernel_spmd

F32 = mybir.dt.float32
F32R = mybir.dt.float32r



B, S, D, H = 4, 1024, 1024, 16
d_head = D // H  # 64
HPC = 8          # heads per core
JC = HPC * d_head  # 512 columns of W per core

_cached = {}


def build_program(use_mask: bool, loop_n=None):
    nc = bacc.Bacc(None, target_bir_lowering=False, debug=False)

    xqT = nc.dram_tensor("xqT", [D, S], F32, kind="ExternalInput").ap()
    xkT = nc.dram_tensor("xkT", [D, S], F32, kind="ExternalInput").ap()
    xvT = nc.dram_tensor("xvT", [D, S], F32, kind="ExternalInput").ap()
    wqT = nc.dram_tensor("wqT", [D, JC], F32, kind="ExternalInput").ap()
    wkT = nc.dram_tensor("wkT", [D, JC], F32, kind="ExternalInput").ap()
    wvT = nc.dram_tensor("wvT", [D, JC], F32, kind="ExternalInput").ap()
    bq_col = nc.dram_tensor("bq_col", [128, 4], F32, kind="ExternalInput").ap()
    bk_col = nc.dram_tensor("bk_col", [128, 4], F32, kind="ExternalInput").ap()
    bv_bc = nc.dram_tensor("bv_bc", [128, JC], F32, kind="ExternalInput").ap()
    woT = nc.dram_tensor("woT", [D, D], F32, kind="ExternalInput").ap()
    if use_mask:
        pen = nc.dram_tensor("pen", [S, S], F32, kind="ExternalInput").ap()
    out = nc.dram_tensor("out", [JC, D], F32, kind="ExternalOutput").ap()

    with tile.TileContext(nc) as tc:
        with (
            tc.tile_pool(name="big", bufs=4) as big,         # [128,4096] 2MB tiles
            tc.tile_pool(name="wp", bufs=2) as wp,           # [128,4096] tiles
            tc.tile_pool(name="qt", bufs=4) as qt_p,
            tc.tile_pool(name="kt", bufs=4) as kt_p,
            tc.tile_pool(name="va", bufs=8) as va_p,
            tc.tile_pool(name="wT", bufs=6) as wT_p,
            tc.tile_pool(name="lh", bufs=2) as lh_p,
            tc.tile_pool(name="outp", bufs=2) as outp,
            tc.tile_pool(name="small", bufs=6) as smallp,
            tc.tile_pool(name="psA", bufs=2, space="PSUM") as psA,
            tc.tile_pool(name="psB", bufs=4, space="PSUM") as psB,
        ):
            from contextlib import ExitStack
            _stk = ExitStack()
            if loop_n is not None:
                _stk.enter_context(tc.For_i(0, loop_n))
            # ---- Q / K projections -> QT/KT [j, s] ----
            def load_wide(dram, pool, tag, ncols, rows_per_tile, n,
                          eng=None):
                # [128, rows_per_tile*ncols] tiles: row-blocks a interleaved
                # along free so each DRAM matrix needs few big DMAs
                a = rows_per_tile
                # default: SWDGE casts fp32 -> fp32r in-flight; with
                # bitcast=True the host pre-rounded the data and the load
                # rides the (otherwise idle) SP HWDGE ring in parallel
                bitcast = eng is not None
                eng = eng or nc.gpsimd
                ts = []
                for i in range(n):
                    t = pool.tile([128, a * ncols], F32R, tag=tag, name=tag)
                    src_ap = (dram[i * a * 128:(i + 1) * a * 128, :]
                              .rearrange("(a p) s -> p a s", a=a))
                    if bitcast:
                        src_ap = src_ap.bitcast(F32R)
                    eng.dma_start(t[:], src_ap)
                    ts.append(t)
                return lambda dt: ts[dt // a][:, (dt % a) * ncols:
                                              (dt % a + 1) * ncols]

            pen_t = None
            if use_mask:
                # binary keep-mask as 2 persistent quad tiles (fallback
                # path: slow but correct; the fast path never loads these)
                pen_t = load_wide(pen, big, "x", S, 4, 2)

            def proj_jt(wt, xt, bias_sb, dst, jt, use_psA=False):
                if use_psA:
                    big_ps = psA.tile([128, 1024], F32, tag="sc", name="kps")
                for st in range(2):
                    if use_psA:
                        ps = big_ps[:, st * 512:(st + 1) * 512]
                    else:
                        ps = psB.tile([128, 512], F32, tag="ps1", name="ps")[:]
                    for dt in range(8):
                        nc.tensor.matmul(
                            ps,
                            lhsT=wt(dt)[:, jt * 128:(jt + 1) * 128],
                            rhs=xt(dt)[:, st * 512:(st + 1) * 512],
                            start=(dt == 0),
                            stop=(dt == 7),
                        )
                    nc.vector.tensor_scalar_add(
                        dst[jt][:, st * 512:(st + 1) * 512], ps,
                        bias_sb[:, jt:jt + 1],
                    )

            def proj_qk(wt, xt, bias_sb, dst_pool):
                dst = [dst_pool.tile([128, S], F32R, tag="dst", name="dst") for _ in range(4)]
                for jt in range(4):
                    proj_jt(wt, xt, bias_sb, dst, jt)
                return dst

            warm = smallp.tile([1, 8], F32, tag="warm", bufs=1)
            nc.vector.memset(warm[:], 0.0)
            nc.scalar.activation(warm[:], warm[:],
                                 mybir.ActivationFunctionType.Exp)

            wt_q = load_wide(wqT, wp, "w", JC, 8, 1, eng=nc.sync)
            xt_q = load_wide(xqT, big, "x", S, 4, 2)
            wt_k = load_wide(wkT, wp, "w", JC, 8, 1, eng=nc.sync)
            xt_k = load_wide(xkT, big, "x", S, 4, 2, eng=nc.sync)
            # biases ride the gpsimd ring so they never delay the K stream
            bq_sb = smallp.tile([128, 4], F32, tag="bias", bufs=2)
            nc.gpsimd.dma_start(bq_sb[:], bq_col[:])
            bk_sb = smallp.tile([128, 4], F32, tag="bias", bufs=2)
            nc.gpsimd.dma_start(bk_sb[:], bk_col[:])
            bv_sb = smallp.tile([128, JC], F32, tag="biasr", bufs=1)
            nc.gpsimd.dma_start(bv_sb[:], bv_bc[:])
            QT = proj_qk(wt_q, xt_q, bq_sb, qt_p)
            KT = proj_qk(wt_k, xt_k, bk_sb, kt_p)

            # ---- V projection -> V_aug [s, 8*65] (65th col per head = 1.0)
            wvt = load_wide(wvT, wp, "w", JC, 8, 1, eng=nc.sync)
            xvt = load_wide(xvT, big, "x", S, 4, 2)  # Pool ring: balances BW
            VA = []
            for st in range(8):
                ps = psB.tile([128, 512], F32, tag="ps1")
                for dt in range(8):
                    nc.tensor.matmul(
                        ps[:],
                        lhsT=xvt(dt)[:, st * 128:(st + 1) * 128],
                        rhs=wvt(dt),
                        start=(dt == 0),
                        stop=(dt == 7),
                    )
                va = va_p.tile([128, 8 * 65], F32R)
                nc.vector.memset(va[:].bitcast(F32), 1.0)
                nc.vector.tensor_tensor(
                    va[:].rearrange("p (h c) -> p h c", h=8)[:, :, 0:64],
                    ps[:].rearrange("p (h c) -> p h c", h=8),
                    bv_sb[:].rearrange("p (h c) -> p h c", h=8),
                    op=mybir.AluOpType.add,
                )
                VA.append(va)

            # woT tiles (reuse big pool slots released by x tiles)
            wo_t = load_wide(woT, big, "x", D, 4, 2)


            def QT_perm(hl, qch):
                # rhs [64, 512] with q in s16-major order:
                # col j reads s = q16*16 + s16, s16 = qch*8 + j//64, q16 = j%64
                tile_ = QT[hl // 2]
                po = (hl % 2) * 64
                ap = tile_[po:po + 64, :].rearrange("p (q s) -> p s q", s=16)
                return ap[:, qch * 8:(qch + 1) * 8, :]

            def KT_ap(hl, kt):
                tile_ = KT[hl // 2]
                po = (hl % 2) * 64
                return tile_[po:po + 64, kt * 128:(kt + 1) * 128]

            # ---- attention per head pair ----
            rc = smallp.tile([64, 1024], F32, tag="rc", bufs=1)
            rcb = smallp.tile([64, 1024], F32, tag="rcb", bufs=1)
            nc.vector.memset(rc[:], 1.0)  # rows 1-63 only feed the bcast AP

            PV_LAG = 2  # kt-steps the PV matmuls trail scores/exp

            def attention(p, hook_norm=None, hook_fp=None, lag=None):
                lag = PV_LAG if lag is None else lag
                hA, hB = 2 * p, 2 * p + 1
                pv = {}
                wstash = {}
                for step in range(8 + lag):
                    if step == 1 and hook_norm is not None:
                        hook_norm()
                    if step == 4 and hook_fp is not None:
                        hook_fp()
                    if step < 8:
                        kt = step
                        scA = psA.tile([128, 1024], F32, tag="sc")
                        scB = psA.tile([128, 1024], F32, tag="sc")
                        # interleave A/B: row-groups 0-63/64-127 overlap on PE
                        for qch in range(2):
                            nc.tensor.matmul(
                                scA[:, qch * 512:(qch + 1) * 512],
                                lhsT=KT_ap(hA, kt),
                                rhs=QT_perm(hA, qch),
                                start=True, stop=True,
                            )
                            nc.tensor.matmul(
                                scB[:, qch * 512:(qch + 1) * 512],
                                lhsT=KT_ap(hB, kt),
                                rhs=QT_perm(hB, qch),
                                start=True, stop=True,
                            )
                        wA = wT_p.tile([128, 1024], F32R, tag="wT")
                        wB = wT_p.tile([128, 1024], F32R, tag="wT")
                        nc.scalar.activation(wA[:], scA[:],
                                             mybir.ActivationFunctionType.Exp,
                                             scale=0.125)
                        nc.scalar.activation(wB[:], scB[:],
                                             mybir.ActivationFunctionType.Exp,
                                             scale=0.125)
                        if use_mask:
                            # multiply by the 0/1 keep-mask (pen[k, q]) with
                            # the same s16-major q permutation as wT columns
                            pap = pen_t(kt).rearrange("p (q s) -> p s q", s=16)
                            for w_ in (wA, wB):
                                nc.vector.tensor_tensor(
                                    w_[:].rearrange("p (s q) -> p s q", s=16),
                                    w_[:].rearrange("p (s q) -> p s q", s=16),
                                    pap, op=mybir.AluOpType.mult,
                                )
                        wstash[kt] = (wA, wB)
                    if step >= lag:
                        kt = step - lag
                        wA, wB = wstash.pop(kt)
                        for i, (hl, wt_, qch) in enumerate(
                            [(hA, wA, 0), (hB, wB, 0), (hA, wA, 1), (hB, wB, 1)]
                        ):
                            if kt == 0:
                                pv[i] = psB.tile([65, 512], F32, tag="ps1", name="pv")
                            nc.tensor.matmul(
                                pv[i][:],
                                lhsT=VA[kt][:, hl * 65:hl * 65 + 65],
                                rhs=wt_[:, qch * 512:(qch + 1) * 512],
                                start=(kt == 0), stop=(kt == 7),
                            )
                return pv

            def tail_norm(p, pv):
                hA, hB = 2 * p, 2 * p + 1
                # normalize + shuffle into final-projection lhsT layout
                lh = lh_p.tile([128, 1024], F32R)
                for hloc, hl in enumerate((hA, hB)):
                    for qch in range(2):
                        i = hloc + 2 * qch
                        nc.vector.reciprocal(
                            rc[0:1, qch * 512:(qch + 1) * 512], pv[i][64:65, :])
                    nc.gpsimd.partition_broadcast(rcb[:], rc[:])
                    rcv = rcb[:].rearrange("p (s q) -> p s q", s=16)
                    for qch in range(2):
                        i = hloc + 2 * qch
                        src = pv[i][0:64, :].rearrange("p (s q) -> p s q", s=8)
                        for par, off in ((0, 0), (1, 64)):  # even/odd s16
                            # lh layout: [part, (ct 8)(head 2)(q16 64)] so the
                            # final matmul's lhsT tile ct is one contiguous
                            # 128-col block (walrus: stationary AP needs a
                            # single free dim)
                            dst = lh[off:off + 64, :].rearrange(
                                "p (c m) -> p c m", c=8
                            )[:, qch * 4:(qch + 1) * 4,
                              hloc * 64:(hloc + 1) * 64]
                            nc.vector.tensor_tensor(
                                dst,
                                src[:, par::2, :],
                                rcv[:, qch * 8 + par:qch * 8 + 8:2, :],
                                op=mybir.AluOpType.mult,
                            )

                return lh

            def tail_fp(p, lh):
                # final projection: out rows p*128 .. p*128+128
                ob = outp.tile([128, 1024], F32)
                for ot in range(2):
                    fp = psB.tile([128, 512], F32, tag="ps1")
                    for ct in range(8):
                        nc.tensor.matmul(
                            fp[:],
                            lhsT=lh[:, ct * 128:(ct + 1) * 128],
                            rhs=wo_t(ct)[:, ot * 512:(ot + 1) * 512],
                            start=(ct == 0), stop=(ct == 7),
                        )
                    nc.vector.tensor_copy(
                        ob[:, ot * 512:(ot + 1) * 512], fp[:])
                nc.sync.dma_start(out[p * 128:(p + 1) * 128, :], ob[:])

            # software-pipeline: emit pair p's attention before pair p-1's
            # norm/final so PE gap-fills the ACT-paced exp phase
            pending = None
            for p in range(4):
                hn = hf = None
                if pending is not None:
                    pp, ppv = pending
                    box = {}

                    def hn(pp=pp, ppv=ppv, box=box):
                        box["lh"] = tail_norm(pp, ppv)

                    def hf(pp=pp, box=box):
                        tail_fp(pp, box["lh"])

                pv = attention(p, hn, hf)
                pending = (p, pv)
            pp, ppv = pending
            tail_fp(pp, tail_norm(pp, ppv))
            _stk.close()

    nc.compile()
    return nc


def _round_f32r(a):
    """Round fp32 to the f32r (TF32-like, 10-bit mantissa) grid, RNE."""
    u = np.ascontiguousarray(a, np.float32).view(np.uint32)
    u = (u + 0x1000 + ((u >> 13) & 1)) & np.uint32(0xFFFFE000)
    return u.view(np.float32)


def make_in_maps(query, key, value, mask, Wq, bq, Wk, bk, Wv, bv, Wo,
                 pen_b=None):
    woT = _round_f32r(Wo.T)
    maps = []
    for c in range(8):
        b, hf = c // 2, c % 2
        sl = slice(hf * JC, (hf + 1) * JC)
        m = {
            "xqT": _round_f32r(query[b].T),
            "xkT": _round_f32r(key[b].T),
            "xvT": _round_f32r(value[b].T),
            "wqT": _round_f32r(Wq[sl].T),
            "wkT": _round_f32r(Wk[sl].T),
            "wvT": _round_f32r(Wv[sl].T),
            "bq_col": np.ascontiguousarray(bq[sl].reshape(4, 128).T),
            "bk_col": np.ascontiguousarray(bk[sl].reshape(4, 128).T),
            "bv_bc": np.ascontiguousarray(
                np.broadcast_to(bv[sl].reshape(1, JC), (128, JC))),
            "woT": woT,
        }
        if pen_b is not None:
            m["pen"] = pen_b[b]
        maps.append(m)
    return maps


def kernel(query, key, value, mask, Wq, bq, Wk, bk, Wv, bv, Wo):
    query = np.asarray(query, np.float32)
    key = np.asarray(key, np.float32)
    value = np.asarray(value, np.float32)
    mask = np.asarray(mask, np.float32)

    m2d = mask[0]  # [B, S, 64]
    mm = np.stack([m2d[b] @ m2d[b].T for b in range(B)])  # [B, S, S]
    use_mask = bool((mm == 0).any())
    pen_b = None
    if use_mask:
        pen_b = np.where(mm == 0, np.float32(0.0), np.float32(1.0))
        pen_b = np.ascontiguousarray(pen_b, np.float32)

    if use_mask not in _cached:
        _cached[use_mask] = build_program(use_mask)
    nc = _cached[use_mask]

    in_maps = make_in_maps(query, key, value, mask,
                           np.asarray(Wq, np.float32), np.asarray(bq, np.float32),
                           np.asarray(Wk, np.float32), np.asarray(bk, np.float32),
                           np.asarray(Wv, np.float32), np.asarray(bv, np.float32),
                           np.asarray(Wo, np.float32), pen_b)
    res = run_bass_kernel_spmd(nc, in_maps, list(range(8)))

    out = np.empty((B, S, D), np.float32)
    for c in range(8):
        b, hf = c // 2, c % 2
        out[b, hf * JC:(hf + 1) * JC, :] = res.results[c]["out"]
    return out



# revision 5
# speedup vs baseline: 1.1222x; 1.1222x over previous
"""Trainium2 Bass kernel v3: v2 + fine-grained tail scheduling.

Changes vs v2:
  - prev-pair tail (norm + final projection + store) is a work queue spread
    across the next pair's attention steps, per-(head,qch) granularity:
    recip -> half-broadcast [64,512] -> 4 mults -> fp ct-pairs, so PE's fp
    matmuls become ready step by step instead of waiting the full norm.
  - wo tiles allocated from the x-chunk pool: the WAR dependency on old x
    slots delays the wo DMAs out of the startup window where they stole
    bandwidth from the projection feed.
  - off DMA gated behind a 1-element memset (WAW dep) emitted after K proj.
  - K-proj drains split DVE/ACT so KT jt0 is ready sooner for pair-0 scores.
  - per-head rc/rcb tiles decouple the two heads' norm chains.
"""

import numpy as np
import ml_dtypes

import concourse.bass as bass
import concourse.bacc as bacc
import concourse.tile as tile
from concourse import mybir
from concourse.bass_utils import run_bass_kernel_spmd

F32 = mybir.dt.float32
BF16 = mybir.dt.bfloat16

B, S, D, H = 4, 1024, 1024, 16
d_head = D // H
HPC = 8
JC = HPC * d_head

_cached = {}


def build_program(use_mask: bool, loop_n=None, lag=4, last_lag=2, per_step=6):
    nc = bacc.Bacc(None, target_bir_lowering=False, debug=False)

    xqT = nc.dram_tensor("xqT", [D, S], BF16, kind="ExternalInput").ap()
    xkT = nc.dram_tensor("xkT", [D, S], BF16, kind="ExternalInput").ap()
    xvT = nc.dram_tensor("xvT", [D, S], BF16, kind="ExternalInput").ap()
    wqT = nc.dram_tensor("wqT", [D, JC], BF16, kind="ExternalInput").ap()
    wkT = nc.dram_tensor("wkT", [D, JC], BF16, kind="ExternalInput").ap()
    wvT = nc.dram_tensor("wvT", [D, JC], BF16, kind="ExternalInput").ap()
    bq_col = nc.dram_tensor("bq_col", [128, 4], F32, kind="ExternalInput").ap()
    woT = nc.dram_tensor("woT", [D, D], BF16, kind="ExternalInput").ap()
    off = nc.dram_tensor("off", [128, 4096], BF16, kind="ExternalInput").ap()
    if use_mask:
        pen = nc.dram_tensor("pen", [S, S], BF16, kind="ExternalInput").ap()
    out = nc.dram_tensor("out", [JC, D], F32, kind="ExternalOutput").ap()

    with tile.TileContext(nc) as tc:
        with (
            tc.tile_pool(name="xp", bufs=10) as xp,
            tc.tile_pool(name="wp", bufs=8) as wp,
            tc.tile_pool(name="qt", bufs=4) as qt_p,
            tc.tile_pool(name="kt", bufs=4) as kt_p,
            tc.tile_pool(name="va", bufs=8) as va_p,
            tc.tile_pool(name="wT", bufs=10) as wT_p,
            tc.tile_pool(name="lh", bufs=2) as lh_p,
            tc.tile_pool(name="outp", bufs=2) as outp,
            tc.tile_pool(name="small", bufs=8) as smallp,
            tc.tile_pool(name="pen", bufs=8) as pen_p,
            tc.tile_pool(name="psA", bufs=2, space="PSUM") as psA,
            tc.tile_pool(name="psB", bufs=4, space="PSUM") as psB,
        ):
            from contextlib import ExitStack
            _stk = ExitStack()
            if loop_n is not None:
                _stk.enter_context(tc.For_i(0, loop_n))

            warm = smallp.tile([1, 8], F32, tag="warm", bufs=1)
            nc.vector.memset(warm[:], 0.0)
            nc.scalar.activation(warm[:], warm[:],
                                 mybir.ActivationFunctionType.Exp)

            def load_x(dram):
                ts = []
                for i in range(8):
                    t = xp.tile([128, 1024], BF16, tag="x", name="x")
                    nc.gpsimd.dma_start(t[:], dram[i * 128:(i + 1) * 128, :])
                    ts.append(t)
                return ts

            def load_w(dram):
                ts = []
                for i in range(8):
                    t = wp.tile([128, 512], BF16, tag="w", name="w")
                    nc.sync.dma_start(t[:], dram[i * 128:(i + 1) * 128, :])
                    ts.append(t)
                return lambda dt: ts[dt][:]

            wt_q = load_w(wqT)
            xt_q = load_x(xqT)
            wt_k = load_w(wkT)
            xt_k = load_x(xkT)
            wt_v = load_w(wvT)
            xt_v = load_x(xvT)

            bq_sb = smallp.tile([128, 4], F32, tag="bias", bufs=1)
            nc.gpsimd.dma_start(bq_sb[:], bq_col[:])

            pen_t = None
            if use_mask:
                pts = []
                for i in range(8):
                    t = pen_p.tile([128, 1024], BF16, tag="pen", name="pen")
                    nc.gpsimd.dma_start(t[:], pen[i * 128:(i + 1) * 128, :])
                    pts.append(t)
                pen_t = lambda kt: pts[kt]

            def proj_qk(wt, xts, dst_pool, drain):
                bigs = [psA.tile([128, 1024], F32, tag="sc", name="prj")
                        for _ in range(2)]
                sms = [psB.tile([128, 512], F32, tag="ps1", name="prj")
                       for _ in range(4)]

                def pview(jt, st):
                    if jt < 2:
                        return bigs[jt][:, st * 512:(st + 1) * 512]
                    return sms[(jt - 2) * 2 + st][:]

                for dt in range(8):
                    for jt in range(4):
                        for st in range(2):
                            nc.tensor.matmul(
                                pview(jt, st),
                                lhsT=wt(dt)[:, jt * 128:(jt + 1) * 128],
                                rhs=xts[dt][:, st * 512:(st + 1) * 512],
                                start=(dt == 0), stop=(dt == 7),
                            )
                dst = [dst_pool.tile([128, S], BF16, tag="dst", name="dst")
                       for _ in range(4)]
                for jt in range(4):
                    for st in range(2):
                        drain(dst[jt][:, st * 512:(st + 1) * 512],
                              pview(jt, st), jt)
                return dst

            def q_drain(dst, ps, jt):
                # split DVE/ACT so K proj's psum-slot reuse isn't serialized
                # behind one engine's drain queue
                if jt in (0, 1):
                    nc.vector.tensor_scalar_add(dst, ps, bq_sb[:, jt:jt + 1])
                else:
                    nc.scalar.activation(dst, ps,
                                         mybir.ActivationFunctionType.Identity,
                                         bias=bq_sb[:, jt:jt + 1])

            def k_drain(dst, ps, jt):
                # jt0/jt1 on ACT (idle then) so KT is ready for early pairs
                if jt in (0, 1):
                    nc.scalar.activation(dst, ps,
                                         mybir.ActivationFunctionType.Copy)
                else:
                    nc.vector.tensor_copy(dst, ps)

            QT = proj_qk(wt_q, xt_q, qt_p, q_drain)
            KT = proj_qk(wt_k, xt_k, kt_p, k_drain)

            # wo from the x pool: WAR dep on old x slots delays these DMAs
            # past the projection feed window
            wo_ts = []
            for i in range(8):
                t = xp.tile([128, 1024], BF16, tag="x", name="wo")
                nc.sync.dma_start(t[:], woT[i * 128:(i + 1) * 128, :])
                wo_ts.append(t)
            off_sb = smallp.tile([128, 4096], BF16, tag="off", bufs=1)
            # true-data WAW gate: the copy reads KT (ready ~K-proj end), so
            # the off DMA can't steal bandwidth from the projection feed
            nc.vector.tensor_copy(off_sb[0:1, 0:1], KT[0][0:1, 0:1])
            nc.sync.dma_start(off_sb[:], off[:])

            # ---- V projection -> V_aug [s, 8*65] (65th col per head = 1.0)
            VA = []
            for st in range(8):
                ps = psB.tile([128, 512], F32, tag="ps1", name="vprj")
                for dt in range(8):
                    nc.tensor.matmul(
                        ps[:],
                        lhsT=xt_v[dt][:, st * 128:(st + 1) * 128],
                        rhs=wt_v(dt),
                        start=(dt == 0), stop=(dt == 7),
                    )
                va = va_p.tile([128, 8 * 65], BF16, name="va")
                vv = va[:].rearrange("p (h c) -> p h c", h=8)
                nc.vector.memset(vv[:, :, 64:65], 1.0)
                nc.vector.tensor_copy(
                    vv[:, :, 0:64],
                    ps[:].rearrange("p (h c) -> p h c", h=8))
                VA.append(va)

            def QT_perm(hl, qch):
                tile_ = QT[hl // 2]
                po = (hl % 2) * 64
                ap = tile_[po:po + 64, :].rearrange("p (q s) -> p s q", s=16)
                return ap[:, qch * 8:(qch + 1) * 8, :]

            def KT_ap(hl, kt):
                tile_ = KT[hl // 2]
                po = (hl % 2) * 64
                return tile_[po:po + 64, kt * 128:(kt + 1) * 128]

            # per-head rc/rcb; rows 1-63 of rc only feed the bcast AP
            rcs, rcbs = [], []
            for hloc in range(2):
                rc = smallp.tile([64, 1024], F32, tag=f"rc{hloc}", bufs=1,
                                 name="rc")
                nc.vector.memset(rc[:], 1.0)
                rcs.append(rc)
                rcb = smallp.tile([64, 1024], F32, tag=f"rcb{hloc}", bufs=1,
                                  name="rcb")
                rcbs.append(rcb)

            def make_tail(pp, ppv):
                """Work queue for pair pp's normalize + final projection."""
                W = []
                st8 = {}

                def alloc_lh():
                    st8["lh"] = lh_p.tile([128, 1024], BF16, name="lh")

                def recip(hloc, qch):
                    i = hloc + 2 * qch
                    nc.vector.reciprocal(
                        rcs[hloc][0:1, qch * 512:(qch + 1) * 512],
                        ppv[i][64:65, :])

                def bcast(hloc, qch):
                    nc.gpsimd.partition_broadcast(
                        rcbs[hloc][:, qch * 512:(qch + 1) * 512],
                        rcs[hloc][:, qch * 512:(qch + 1) * 512])

                def mult(hloc, qch, par):
                    i = hloc + 2 * qch
                    lh = st8["lh"]
                    src = ppv[i][0:64, :].rearrange("p (s q) -> p s q", s=8)
                    rcv = rcbs[hloc][:].rearrange("p (s q) -> p s q", s=16)
                    dst = lh[par * 64:par * 64 + 64, :].rearrange(
                        "p (c m) -> p c m", c=8
                    )[:, qch * 4:(qch + 1) * 4, hloc * 64:(hloc + 1) * 64]
                    nc.vector.tensor_tensor(
                        dst, src[:, par::2, :],
                        rcv[:, qch * 8 + par:qch * 8 + 8:2, :],
                        op=mybir.AluOpType.mult)

                def fp(ot, j):
                    if j == 0:
                        st8[f"fp{ot}"] = psB.tile([128, 512], F32, tag="ps1",
                                                  name="fp")
                    for ct in (2 * j, 2 * j + 1):
                        nc.tensor.matmul(
                            st8[f"fp{ot}"][:],
                            lhsT=st8["lh"][:, ct * 128:(ct + 1) * 128],
                            rhs=wo_ts[ct][:, ot * 512:(ot + 1) * 512],
                            start=(ct == 0), stop=(ct == 7))

                def drain(ot):
                    if ot == 0:
                        st8["ob"] = outp.tile([128, 1024], F32, name="ob")
                    ob = st8["ob"]
                    nc.vector.tensor_tensor(
                        ob[:, ot * 512:(ot + 1) * 512], st8[f"fp{ot}"][:],
                        off_sb[:, pp * 1024 + ot * 512:
                               pp * 1024 + (ot + 1) * 512],
                        op=mybir.AluOpType.add)
                    nc.sync.dma_start(
                        out[pp * 128:(pp + 1) * 128,
                            ot * 512:(ot + 1) * 512],
                        ob[:, ot * 512:(ot + 1) * 512])

                W.append(alloc_lh)
                for qch in range(2):
                    for hloc in range(2):
                        W.append(lambda h=hloc, q=qch: recip(h, q))
                    for hloc in range(2):
                        W.append(lambda h=hloc, q=qch: bcast(h, q))
                    for hloc in range(2):
                        for par in range(2):
                            W.append(lambda h=hloc, q=qch, p_=par:
                                     mult(h, q, p_))
                for ot in range(2):
                    for j in range(4):
                        W.append(lambda o=ot, j_=j: fp(o, j_))
                    W.append(lambda o=ot: drain(o))
                return W

            def attention(p, tail, lag_):
                hA, hB = 2 * p, 2 * p + 1
                pv = {}
                wstash = {}
                consumed = 0
                for step in range(8 + lag_):
                    want = min(len(tail), (step + 1) * per_step)
                    while consumed < want:
                        tail[consumed]()
                        consumed += 1
                    if step < 8:
                        kt = step
                        scA = psA.tile([128, 1024], F32, tag="sc")
                        scB = psA.tile([128, 1024], F32, tag="sc")
                        for qch in range(2):
                            nc.tensor.matmul(
                                scA[:, qch * 512:(qch + 1) * 512],
                                lhsT=KT_ap(hA, kt), rhs=QT_perm(hA, qch),
                                start=True, stop=True)
                            nc.tensor.matmul(
                                scB[:, qch * 512:(qch + 1) * 512],
                                lhsT=KT_ap(hB, kt), rhs=QT_perm(hB, qch),
                                start=True, stop=True)
                        wA = wT_p.tile([128, 1024], BF16, tag="wT")
                        wB = wT_p.tile([128, 1024], BF16, tag="wT")
                        nc.scalar.activation(wA[:], scA[:],
                                             mybir.ActivationFunctionType.Exp,
                                             scale=0.125)
                        nc.scalar.activation(wB[:], scB[:],
                                             mybir.ActivationFunctionType.Exp,
                                             scale=0.125)
                        if use_mask:
                            pap = pen_t(kt).rearrange("p (q s) -> p s q", s=16)
                            for w_ in (wA, wB):
                                nc.vector.tensor_tensor(
                                    w_[:].rearrange("p (s q) -> p s q", s=16),
                                    w_[:].rearrange("p (s q) -> p s q", s=16),
                                    pap, op=mybir.AluOpType.mult)
                        wstash[kt] = (wA, wB)
                    if step >= lag_:
                        kt = step - lag_
                        wA, wB = wstash.pop(kt)
                        for i, (hl, wt_, qch) in enumerate(
                            [(hA, wA, 0), (hB, wB, 0), (hA, wA, 1), (hB, wB, 1)]
                        ):
                            if kt == 0:
                                pv[i] = psB.tile([65, 512], F32, tag="ps1",
                                                 name="pv")
                            nc.tensor.matmul(
                                pv[i][:],
                                lhsT=VA[kt][:, hl * 65:hl * 65 + 65],
                                rhs=wt_[:, qch * 512:(qch + 1) * 512],
                                start=(kt == 0), stop=(kt == 7))
                while consumed < len(tail):
                    tail[consumed]()
                    consumed += 1
                return pv

            pending = None
            for p in range(4):
                tail = [] if pending is None else make_tail(*pending)
                pv = attention(p, tail, lag if p < 3 else last_lag)
                pending = (p, pv)
            for fn in make_tail(*pending):
                fn()
            _stk.close()

    nc.compile()
    return nc


def _bf16(a):
    return np.asarray(a, np.float32).astype(ml_dtypes.bfloat16)


def make_in_maps(query, key, value, mask, Wq, bq, Wk, bk, Wv, bv, Wo,
                 pen_b=None):
    woT = _bf16(Wo.T)
    Wo32 = np.asarray(Wo, np.float32)
    maps = []
    for c in range(8):
        b, hf = c // 2, c % 2
        sl = slice(hf * JC, (hf + 1) * JC)
        bv_sl = np.asarray(bv, np.float32)[sl]
        offvec = np.stack([
            np.tile(bv_sl[h * 64:(h + 1) * 64], 16) @ Wo32.T
            for h in range(8)
        ])  # [8, 1024]
        off_arr = np.empty((128, 4096), np.float32)
        for p in range(4):
            for part in range(128):
                off_arr[part, p * 1024:(p + 1) * 1024] = \
                    offvec[2 * p + part // 64]
        m = {
            "xqT": _bf16(query[b].T),
            "xkT": _bf16(key[b].T),
            "xvT": _bf16(value[b].T),
            "wqT": _bf16(Wq[sl].T),
            "wkT": _bf16(Wk[sl].T),
            "wvT": _bf16(Wv[sl].T),
            "bq_col": np.ascontiguousarray(
                np.asarray(bq, np.float32)[sl].reshape(4, 128).T),
            "woT": woT,
            "off": off_arr.astype(ml_dtypes.bfloat16),
        }
        if pen_b is not None:
            m["pen"] = _bf16(pen_b[b])
        maps.append(m)
    return maps


def kernel(query, key, value, mask, Wq, bq, Wk, bk, Wv, bv, Wo):
    query = np.asarray(query, np.float32)
    key = np.asarray(key, np.float32)
    value = np.asarray(value, np.float32)
    mask = np.asarray(mask, np.float32)

    m2d = mask[0]
    mm = np.stack([m2d[b] @ m2d[b].T for b in range(B)])
    use_mask = bool((mm == 0).any())
    pen_b = None
    if use_mask:
        pen_b = np.where(mm == 0, np.float32(0.0), np.float32(1.0))
        pen_b = np.ascontiguousarray(pen_b, np.float32)

    if use_mask not in _cached:
        _cached[use_mask] = build_program(use_mask)
    nc = _cached[use_mask]

    in_maps = make_in_maps(query, key, value, mask,
                           np.asarray(Wq, np.float32), np.asarray(bq, np.float32),
                           np.asarray(Wk, np.float32), np.asarray(bk, np.float32),
                           np.asarray(Wv, np.float32), np.asarray(bv, np.float32),
                           np.asarray(Wo, np.float32), pen_b)
    res = run_bass_kernel_spmd(nc, in_maps, list(range(8)))

    out = np.empty((B, S, D), np.float32)
    for c in range(8):
        b, hf = c // 2, c % 2
        out[b, hf * JC:(hf + 1) * JC, :] = res.results[c]["out"]
    return out
